# revision 41
# baseline (speedup 1.0000x reference)
"""Trainium2 Bass kernel for a 2-layer GATv2 encoder + LayerNorm (ASTGATEncoder).

Strategy (8 NeuronCores, SPMD single NEFF):
  - Rank layout: dst nodes sorted by in-degree into 80 blocks of 128 slots
    (slot = SBUF partition), blocks dealt serpentine to cores so the k-th
    block of every core has a similar max degree R_k (shared control flow).
    Edge (i, r) = slot i's r-th in-edge; pads gather a zero row.
  - |att| is folded into Wl/Wr on the host (prelu is positively
    homogeneous), so the score is sum_f sign(att_f) * prelu(u_f) with
    u = ul_src + ur_dst: a plain Prelu plus one constant signed-head
    matmul. The 1/|att| undo folds into layer-2 weights (for L1) and one
    epilogue multiply (for L2). Exact, no approximation.
  - Per 4-rank batch: one dma_gather slice of ul rows; DVE adds the
    per-block ur tile (row-aligned in rank layout); PE transposes; Prelu
    rides the PSUM->SBUF copy on ScalarE; signed-head matmul -> scores;
    one Exp per batch; per-rank: w transposed back, v = w * ul on DVE,
    identity-lhsT matmul accumulates into PSUM.
  - Denominators: whole-block w tile (masked once per batch) tree-folded
    on DVE at block end.
  - h exchanged via 2 AllGathers (after blocks 7 and 10), ul2 for all
    nodes computed locally, overlapped with remaining L1 blocks.
"""
import os
import sys
import time

sys.path.insert(0, "/opt/trn_rl_repo")

# CoreSim does not implement the Prelu activation; this switches the
# prelu to an equivalent DVE max(z, alpha*z) for simulation runs.
PRELU_ON_DVE = bool(os.environ.get("KERNEL_PRELU_ON_DVE"))

import numpy as np
import ml_dtypes

import concourse.bass as bass
import concourse.bacc as bacc
import concourse.mybir as mybir
import concourse.tile as tile

bf16 = ml_dtypes.bfloat16
F32 = mybir.dt.float32
BF = mybir.dt.bfloat16
I16 = mybir.dt.int16

NCORES = 8
SLOPE = 0.2
EPS = 1e-5
G = 4          # ranks per compute batch
GDMA = 8       # ranks per dma_gather call (1024 idxs = 64 descs/engine)
AG_GROUPS = ((0, 4), (4, 4), (8, 2))   # (start_blk, nblocks) per AllGather


# ----------------------------------------------------------------- host prep

def _wrap_idxs(idx):
    """Flat int array -> [128, ceil(n/16)] int16 SWDGE layout (idx i at
    partition i%16, col i//16, replicated across the 8 groups of 16)."""
    idx = np.asarray(idx)
    n = len(idx)
    cols = (n + 15) // 16
    pad = np.zeros(cols * 16, np.int16)
    pad[:n] = idx.astype(np.int16)
    out = np.zeros((128, cols), np.int16)
    out[:16] = pad.reshape(cols, 16).T
    for g in range(1, 8):
        out[g * 16:(g + 1) * 16] = out[:16]
    return out


class _Prep:
    """All host-side preprocessing derived from edge_index + shapes."""

    def __init__(self, N, E, F_IN, HID, OUT, H, edge_index):
        self.N, self.F_IN, self.HID, self.OUT, self.H = N, F_IN, HID, OUT, H
        ei = np.asarray(edge_index)
        src = np.concatenate([ei[0], np.arange(N, dtype=np.int64)])
        dst = np.concatenate([ei[1], np.arange(N, dtype=np.int64)])
        deg = np.bincount(dst, minlength=N)

        self.NBLK = ((N + NCORES - 1) // NCORES + 127) // 128   # blocks/core
        nbins = NCORES * self.NBLK
        order = np.argsort(-deg, kind="stable")     # nodes by degree desc
        # serpentine deal: block slot k of core c <- global block k*8 + s(c)
        nbin = np.full(N, -1, np.int32)   # node -> (core, blk) packed
        nslot = np.zeros(N, np.int32)
        self.slot_nodes = {}
        for gb in range(nbins):
            k, j = divmod(gb, NCORES)
            c = j if k % 2 == 0 else NCORES - 1 - j
            nodes = order[gb * 128:(gb + 1) * 128]
            nbin[nodes] = c * self.NBLK + k
            nslot[nodes] = np.arange(len(nodes))
            self.slot_nodes[(c, k)] = nodes
        self.nbin, self.nslot = nbin, nslot
        core_of = nbin // self.NBLK
        blk_of = nbin % self.NBLK

        # shared per-slot rank counts R_k (max over cores, padded to G)
        Rk = np.zeros(self.NBLK, np.int64)
        for (c, k), nodes in self.slot_nodes.items():
            if len(nodes):
                Rk[k] = max(Rk[k], deg[nodes].max())
        Rk = np.maximum(Rk, 1)
        self.Rk = ((Rk + G - 1) // G * G).astype(np.int64)
        self.NRANK = int(self.Rk.sum())

        # l2 table row of each node (ul2 table in (core, blk, slot) order)
        self.l2row = core_of.astype(np.int64) * (self.NBLK * 128) \
            + blk_of * 128 + nslot

        # per-(core, blk): [R_k, 128] src table (-1 = pad)
        esl = np.argsort(dst, kind="stable")
        dst_s, src_s = dst[esl], src[esl]
        starts = np.concatenate([[0], np.cumsum(np.bincount(dst_s, minlength=N))])
        XPAD = ((N + 127) // 128) * 128
        self.ZROW1 = XPAD                      # zero row in ul1 table
        self.ZROW2 = nbins * 128               # zero row in ul2 table
        self.idx1 = [None] * NCORES
        self.idx2 = [None] * NCORES
        self.maskT = [None] * NCORES
        # per-gather-chunk rank counts (shared across cores)
        self.chunks = []                       # list of (blk, csize)
        for k in range(self.NBLK):
            r = 0
            while r < self.Rk[k]:
                cs = min(GDMA, self.Rk[k] - r)
                self.chunks.append((k, int(cs)))
                r += cs
        for c in range(NCORES):
            i1, i2, mk = [], [], []
            for k in range(self.NBLK):
                R = int(self.Rk[k])
                nodes = self.slot_nodes[(c, k)]
                s1 = np.full((R, 128), self.ZROW1, np.int64)
                s2 = np.full((R, 128), self.ZROW2, np.int64)
                m = np.zeros((R, 128), np.float32)
                for sl, n in enumerate(nodes):
                    d = int(deg[n])
                    e0 = starts[n]
                    s = src_s[e0:e0 + d]
                    s1[:d, sl] = s
                    s2[:d, sl] = self.l2row[s]
                    m[:d, sl] = 1.0
                i1.append(s1)
                i2.append(s2)
                mk.append(m)
            # wrap idxs per gather chunk
            w1, w2 = [], []
            pos = {k: 0 for k in range(self.NBLK)}
            for (k, cs) in self.chunks:
                r0 = pos[k]
                w1.append(_wrap_idxs(i1[k][r0:r0 + cs].reshape(-1)))
                w2.append(_wrap_idxs(i2[k][r0:r0 + cs].reshape(-1)))
                pos[k] = r0 + cs
            self.idx1[c] = np.concatenate(w1, 1)
            self.idx2[c] = np.concatenate(w2, 1)
            mk_flat = np.concatenate([m.reshape(-1) for m in mk])
            self.maskT[c] = np.broadcast_to(
                mk_flat, (4, self.NRANK * 128)).astype(bf16).copy()


# --------------------------------------------------------------- device build

def _build_nc(p):
    N, F_IN, HID, OUT, H = p.N, p.F_IN, p.HID, p.OUT, p.H
    NBLK = p.NBLK
    XPAD = ((N + 127) // 128) * 128
    NOWN = NBLK * 128
    KIN = F_IN // 128
    K1 = HID // 128
    KF = {1: HID // 128, 2: OUT // 128}
    NROW2 = NCORES * NOWN

    nc = bacc.Bacc("TRN2", target_bir_lowering=False, debug=False,
                   num_devices=NCORES)
    # ---- external inputs
    xT = nc.dram_tensor("xT", [F_IN, XPAD], BF, kind="ExternalInput")
    xownT = nc.dram_tensor("xownT", [F_IN, NOWN], BF, kind="ExternalInput")
    WlT1 = nc.dram_tensor("WlT1", [F_IN, HID], BF, kind="ExternalInput")
    WrT1 = nc.dram_tensor("WrT1", [F_IN, HID], BF, kind="ExternalInput")
    WlT2 = nc.dram_tensor("WlT2", [HID, OUT], BF, kind="ExternalInput")
    WrT2 = nc.dram_tensor("WrT2", [HID, OUT], BF, kind="ExternalInput")
    S1d = nc.dram_tensor("S1d", [HID, H], BF, kind="ExternalInput")
    S2d = nc.dram_tensor("S2d", [OUT, H], BF, kind="ExternalInput")
    b1rep = nc.dram_tensor("b1rep", [128, HID], F32, kind="ExternalInput")
    b2rep = nc.dram_tensor("b2rep", [128, OUT], F32, kind="ExternalInput")
    blr1 = nc.dram_tensor("blr1", [1, HID], BF, kind="ExternalInput")
    brr1 = nc.dram_tensor("brr1", [1, HID], BF, kind="ExternalInput")
    inva2r = nc.dram_tensor("inva2r", [128, OUT], F32, kind="ExternalInput")
    gam = nc.dram_tensor("gam", [128, OUT], F32, kind="ExternalInput")
    bet = nc.dram_tensor("bet", [128, OUT], F32, kind="ExternalInput")
    ident = nc.dram_tensor("ident", [128, 128], BF, kind="ExternalInput")
    identf = nc.dram_tensor("identf", [4, 4], F32, kind="ExternalInput")
    blr2 = nc.dram_tensor("blr2", [1, OUT], BF, kind="ExternalInput")
    brr2 = nc.dram_tensor("brr2", [1, OUT], BF, kind="ExternalInput")
    idx1d = nc.dram_tensor("idx1d", list(p.idx1[0].shape), I16, kind="ExternalInput")
    idx2d = nc.dram_tensor("idx2d", list(p.idx2[0].shape), I16, kind="ExternalInput")
    zr1 = nc.dram_tensor("zr1", [128, HID], BF, kind="ExternalInput")
    zr2 = nc.dram_tensor("zr2", [128, OUT], BF, kind="ExternalInput")
    id8d = nc.dram_tensor("id8d", [4, 8], BF, kind="ExternalInput")
    # ---- outputs
    out_o = nc.dram_tensor("out_o", [NOWN, OUT], F32, kind="ExternalOutput")
    # ---- internal dram
    ULtab = nc.dram_tensor("ULtab", [XPAD + 128, HID], BF)
    UL2S = nc.dram_tensor("UL2S", [NROW2 + 128, OUT], BF)
    hownT = [nc.dram_tensor(f"hownT{gi}", [HID, ng * 128], BF)
             for gi, (_, ng) in enumerate(AG_GROUPS)]
    HST = [nc.dram_tensor(f"HST{gi}", [NCORES * HID, ng * 128], BF,
                          addr_space="Shared")
           for gi, (_, ng) in enumerate(AG_GROUPS)]
    warm_in = nc.dram_tensor("warm_in", [1, 64], F32)
    warm_out = nc.dram_tensor("warm_out", [1, 64], F32, addr_space="Shared")

    RMAX = int(p.Rk.max())

    with tile.TileContext(nc) as tc:
        with (
            tc.tile_pool(name="cons", bufs=1) as cons,
            tc.tile_pool(name="ps_dense", bufs=1, space="PSUM") as psd_pool,
        ):
            # comm-path warmup
            wt = cons.tile([1, 64], F32, tag="warm")
            nc.vector.memset(wt[:], 1.0)
            nc.sync.dma_start(warm_in[:], wt[:])
            nc.gpsimd.collective_compute(
                "AllReduce", mybir.AluOpType.add,
                replica_groups=[list(range(NCORES))],
                ins=[warm_in[:].opt()], outs=[warm_out[:].opt()])
            # ---------------- constants
            wl1 = cons.tile([128, KIN, HID], BF)
            wr1 = cons.tile([128, KIN, HID], BF)
            wl2 = cons.tile([128, K1, OUT], BF)
            wr2 = cons.tile([128, K1, OUT], BF)
            for k in range(KIN):
                nc.sync.dma_start(wl1[:, k, :], WlT1[k * 128:(k + 1) * 128, :])
                nc.sync.dma_start(wr1[:, k, :], WrT1[k * 128:(k + 1) * 128, :])
            for k in range(K1):
                nc.sync.dma_start(wl2[:, k, :], WlT2[k * 128:(k + 1) * 128, :])
                nc.sync.dma_start(wr2[:, k, :], WrT2[k * 128:(k + 1) * 128, :])
            S_t = {}
            for lay, (Sd, Fo) in {1: (S1d, HID), 2: (S2d, OUT)}.items():
                a = cons.tile([128, Fo // 128, H], BF, tag=f"S{lay}")
                for k in range(Fo // 128):
                    nc.sync.dma_start(a[:, k, :], Sd[k * 128:(k + 1) * 128, :])
                S_t[lay] = a
            b1rep_t = cons.tile([128, HID], F32)
            b2rep_t = cons.tile([128, OUT], F32)
            inva2_t = cons.tile([128, OUT], F32)
            gam_t = cons.tile([128, OUT], F32)
            bet_t = cons.tile([128, OUT], F32)
            id_t = cons.tile([128, 128], BF)
            idf_t = cons.tile([4, 4], F32)
            for t, d in [(b1rep_t, b1rep), (b2rep_t, b2rep), (inva2_t, inva2r),
                         (gam_t, gam), (bet_t, bet), (id_t, ident),
                         (idf_t, identf)]:
                nc.sync.dma_start(t[:], d[:])
            ones_t = cons.tile([1, 128], BF)
            nc.vector.memset(ones_t[:], 1.0)
            eps_t = cons.tile([128, 1], F32)
            nc.vector.memset(eps_t[:], EPS)
            blr1_t = cons.tile([1, HID], BF, tag="blr1")
            brr1_t = cons.tile([1, HID], BF, tag="brr1")
            blr2_t = cons.tile([1, OUT], BF, tag="blr2")
            brr2_t = cons.tile([1, OUT], BF, tag="brr2")
            nc.sync.dma_start(blr1_t[:], blr1[:])
            nc.sync.dma_start(brr1_t[:], brr1[:])
            nc.sync.dma_start(blr2_t[:], blr2[:])
            nc.sync.dma_start(brr2_t[:], brr2[:])
            idx1_t = cons.tile(list(p.idx1[0].shape), I16)
            idx2_t = cons.tile(list(p.idx2[0].shape), I16)
            nc.sync.dma_start(idx1_t[:], idx1d[:])
            nc.sync.dma_start(idx2_t[:], idx2d[:])
            # pad rows of the gather tables: -C*sign(att) makes pad scores
            # ~-2000 -> exp == 0, so no masking is needed anywhere
            zt1 = cons.tile([128, HID], BF, tag="zrow1")
            zt2 = cons.tile([128, OUT], BF, tag="zrow2")
            id8_t = cons.tile([4, 8], BF, tag="id8")
            nc.sync.dma_start(zt1[:], zr1[:])
            nc.sync.dma_start(zt2[:], zr2[:])
            nc.sync.dma_start(id8_t[:], id8d[:])
            nc.sync.dma_start(ULtab[XPAD:XPAD + 128, :], zt1[:])
            nc.sync.dma_start(UL2S[NROW2:NROW2 + 128, :], zt2[:])
            # ur tiles resident across the whole kernel
            ur1_t = cons.tile([128, NBLK, HID], BF)
            ur2_t = cons.tile([128, NBLK, OUT], BF)

            # ---------------- dense layer 1 (replicated ul1 + own ur1)
            with tc.tile_pool(name="d1", bufs=1) as d1p, \
                 tc.tile_pool(name="d1w", bufs=3) as d1w:
                xT_t = d1p.tile([128, KIN, XPAD], BF)
                for k in range(KIN):
                    nc.sync.dma_start(xT_t[:, k, :], xT[k * 128:(k + 1) * 128, :])
                xoT_t = d1p.tile([128, KIN, NOWN], BF)
                for k in range(KIN):
                    nc.sync.dma_start(xoT_t[:, k, :], xownT[k * 128:(k + 1) * 128, :])
                # ul1 for all nodes: 4 chunks per PSUM tile, one copy + DMA
                NCH = XPAD // 128
                with tc.tile_pool(name="ps_d1", bufs=2,
                                  space="PSUM") as psd1_pool:
                    for ch0 in range(0, NCH, 4):
                        nch = min(4, NCH - ch0)
                        o = d1w.tile([128, 4, HID], BF, tag="d1o")
                        ps = psd1_pool.tile([128, 4, HID], F32, tag="d1ps")
                        for j in range(nch):
                            lhsT = xT_t[:, :,
                                        (ch0 + j) * 128:(ch0 + j + 1) * 128]
                            for k in range(KIN):
                                nc.tensor.matmul(
                                    ps[:, j, :], lhsT[:, k, :], wl1[:, k, :],
                                    start=(k == 0),
                                    stop=(k == KIN - 1 and not p.use_bias))
                            if p.use_bias:
                                nc.tensor.matmul(ps[:, j, :], ones_t[:],
                                                 blr1_t[:],
                                                 start=False, stop=True)
                        nc.scalar.copy(
                            o[:, :nch, :].rearrange("prt c f -> prt (c f)"),
                            ps[:, :nch, :].rearrange("prt c f -> prt (c f)"))
                        nc.sync.dma_start(
                            ULtab[ch0 * 128:(ch0 + nch) * 128, :].rearrange(
                                "(c prt) f -> prt c f", prt=128),
                            o[:, :nch, :])
                # ur1 for own nodes -> resident SBUF
                for b in range(NBLK):
                    ps = psd_pool.tile([128, HID], F32, tag="dense")
                    lhsT = xoT_t[:, :, b * 128:(b + 1) * 128]
                    for k in range(KIN):
                        nc.tensor.matmul(
                            ps[:], lhsT[:, k, :], wr1[:, k, :],
                            start=(k == 0),
                            stop=(k == KIN - 1 and not p.use_bias))
                    if p.use_bias:
                        nc.tensor.matmul(ps[:], ones_t[:], brr1_t[:],
                                         start=False, stop=True)
                    nc.scalar.copy(ur1_t[:, b, :], ps[:])

            # ---------------- edge phase
            def edge_layer(lay, tab, idx_t, pools, Fo, epilogue):
                """Process all blocks of one layer."""
                (g_pool, ew_pool, eo_pool, wt_pool, pst_pool,
                 psv_pool, pse_pool, pss_pool) = pools
                kf = KF[lay]
                chunk_of_blk = {}
                for ci, (k, cs) in enumerate(p.chunks):
                    chunk_of_blk.setdefault(k, []).append((ci, cs))
                # column offset of each gather chunk in idx table
                idx_off = {}
                off = 0
                for ci, (k, cs) in enumerate(p.chunks):
                    idx_off[ci] = off
                    off += cs * 128 // 16
                rank_base = {}
                rb = 0
                for k in range(NBLK):
                    rank_base[k] = rb
                    rb += int(p.Rk[k])

                for k in range(NBLK):
                    R = int(p.Rk[k])
                    ur = (ur1_t if lay == 1 else ur2_t)[:, k, :]
                    psv = psv_pool.tile([128, Fo], F32, tag="psv")
                    wtile = wt_pool.tile([4, RMAX * 128], BF, tag="wtile")
                    # gather chunks for this block
                    gts = []
                    for (ci, cs) in chunk_of_blk[k]:
                        gt = g_pool.tile([128, GDMA, Fo], BF, tag="g")
                        c0 = idx_off[ci]
                        nc.gpsimd.dma_gather(
                            gt[:, :cs, :], tab[:],
                            idx_t[:, c0:c0 + cs * 128 // 16],
                            cs * 128, cs * 128, Fo)
                        gts.append((gt, cs))
                    # per G-rank batch
                    nb = R // G
                    self_eps = [None]
                    for bi in range(nb):
                        r0 = bi * G
                        gt, cs = gts[r0 // GDMA]
                        gsl = gt[:, r0 % GDMA:r0 % GDMA + G, :]
                        s = ew_pool.tile([128, G, Fo], BF, tag="s")
                        nc.vector.tensor_tensor(
                            s[:], gsl,
                            ur.unsqueeze(1).to_broadcast((128, G, Fo)),
                            op=mybir.AluOpType.add)
                        # PE transposes -> PSUM (GP ranks per psum tile so
                        # two tiles fit in the bank budget for layer 2)
                        e_ps = pse_pool.tile([4, G * 128], F32, tag="e")
                        GP = G if lay == 1 else 2
                        for sub in range(0, G, GP):
                            ntp = GP * kf
                            tT_ps = pst_pool.tile([128, GP * kf * 128], BF,
                                                  tag="tT")
                            for i in range(ntp):
                                c, kk = divmod(i, kf)
                                nc.tensor.matmul(
                                    tT_ps[:, i * 128:(i + 1) * 128],
                                    s[:, sub + c, kk * 128:(kk + 1) * 128],
                                    id_t[:],
                                    is_transpose=True,
                                    start=(i % 8 == 0),
                                    stop=(i % 8 == 7) or (i == ntp - 1))
                            # prelu rides the PSUM->SBUF copy
                            pT = ew_pool.tile([128, GP * kf * 128], BF,
                                              tag="pT")
                            if PRELU_ON_DVE:
                                nc.vector.scalar_tensor_tensor(
                                    pT[:], tT_ps[:], SLOPE, tT_ps[:],
                                    op0=mybir.AluOpType.mult,
                                    op1=mybir.AluOpType.max)
                            else:
                                nc.scalar.activation(
                                    pT[:], tT_ps[:],
                                    mybir.ActivationFunctionType.Prelu,
                                    alpha=SLOPE)
                            # signed head sum -> e_ps [4, GP*128] slice
                            pT3 = pT[:].rearrange(
                                "prt (c kk e) -> prt c kk e", c=GP, kk=kf)
                            esl = e_ps[:, sub * 128:(sub + GP) * 128]
                            for kk in range(kf):
                                nc.tensor.matmul(
                                    esl.rearrange("h (c e) -> h c e", c=GP),
                                    S_t[lay][:, kk, :], pT3[:, :, kk, :],
                                    start=(kk == 0), stop=(kk == kf - 1))
                        # w = exp(e), written straight into the block w tile
                        # (pad edges gathered the -C*sign row -> exp == 0)
                        nc.scalar.activation(
                            wtile[:, r0 * 128:(r0 + G) * 128], e_ps[:],
                            mybir.ActivationFunctionType.Exp)
                        # w back to rows; duplicate into pairs during the
                        # PSUM->SBUF copy so v-mult keeps DVE 2x packed mode
                        wn_ps = pss_pool.tile([128, G, 4], BF, tag="small")
                        for g in range(G):
                            nc.tensor.matmul(
                                wn_ps[:, g, :],
                                wtile[:, (r0 + g) * 128:(r0 + g + 1) * 128],
                                id_t[:4, :4],
                                is_transpose=True,
                                start=(g == 0), stop=(g == G - 1))
                        wn2 = ew_pool.tile([128, G, 4, 2], BF, tag="wns")
                        nc.vector.tensor_copy(
                            wn2[:],
                            wn_ps[:].unsqueeze(3).to_broadcast((128, G, 4, 2)))
                        # v = w (x) ul ; accumulate into psv via identity lhsT
                        d2 = Fo // H // 2
                        v = ew_pool.tile([128, G, Fo], BF, tag="v")
                        nc.vector.tensor_tensor(
                            out=v[:].rearrange(
                                "prt c (h d2 j) -> prt c h d2 j", h=H, j=2),
                            in0=gsl.rearrange(
                                "prt c (h d2 j) -> prt c h d2 j", h=H, j=2),
                            in1=wn2[:].unsqueeze(3).to_broadcast(
                                (128, G, H, d2, 2)),
                            op=mybir.AluOpType.mult)
                        for g in range(G):
                            cc = r0 + g
                            nc.tensor.matmul(psv[:], id_t[:], v[:, g, :],
                                             start=(cc == 0),
                                             stop=(cc == R - 1))
                    # ---- block epilogue: denominators + normalize
                    # tree-fold in 128-col group units (R is a multiple of G)
                    wb = wtile[:, :R * 128]
                    acc = wt_pool.tile([4, RMAX * 128 // 4], F32, tag="dacc")
                    ng = R // 4
                    q = ng * 128
                    nc.gpsimd.tensor_tensor(
                        acc[:, :q], wb[:, :q], wb[:, q:2 * q],
                        op=mybir.AluOpType.add)
                    nc.gpsimd.tensor_tensor(
                        acc[:, :q], acc[:, :q], wb[:, 2 * q:3 * q],
                        op=mybir.AluOpType.add)
                    nc.gpsimd.tensor_tensor(
                        acc[:, :q], acc[:, :q], wb[:, 3 * q:4 * q],
                        op=mybir.AluOpType.add)
                    while ng > 1:
                        h2 = ng // 2
                        nc.gpsimd.tensor_tensor(
                            acc[:, :h2 * 128], acc[:, :h2 * 128],
                            acc[:, h2 * 128:2 * h2 * 128],
                            op=mybir.AluOpType.add)
                        if ng % 2:
                            nc.gpsimd.tensor_tensor(
                                acc[:, :128], acc[:, :128],
                                acc[:, (ng - 1) * 128:ng * 128],
                                op=mybir.AluOpType.add)
                        ng = h2
                    den = ew_pool.tile([4, 128], F32, tag="den")
                    nc.vector.tensor_scalar_add(den[:], acc[:, :128], 1e-30)
                    rec = ew_pool.tile([4, 128], F32, tag="rec")
                    nc.vector.reciprocal(rec[:], den[:])
                    rec_ps = pss_pool.tile([128, 4], F32, tag="small")
                    nc.tensor.matmul(rec_ps[:], rec[:], idf_t[:],
                                     is_transpose=True)
                    recn = ew_pool.tile([128, 4], F32, tag="recn")
                    nc.vector.tensor_copy(recn[:], rec_ps[:])
                    vn = eo_pool.tile([128, Fo], F32, tag="vn")
                    nc.vector.tensor_tensor(
                        out=vn[:].rearrange("prt (h d) -> prt h d", h=H),
                        in0=psv[:].rearrange("prt (h d) -> prt h d", h=H),
                        in1=recn[:].unsqueeze(2).to_broadcast((128, H, Fo // H)),
                        op=mybir.AluOpType.mult)
                    epilogue(k, vn, eo_pool, pss_pool)

            # ---------------- layer 1 epilogue: relu -> h, hT, ur2, AllGather
            def dense_ul2_group(grp, eo_pool):
                """ul2 rows for one gathered group (all cores)."""
                blk0, ng = AG_GROUPS[grp]
                for ccore in range(NCORES):
                    htt = eo_pool.tile([128, K1, max(g[1] for g in AG_GROUPS)
                                        * 128], BF, tag="htt")
                    for k in range(K1):
                        rr = ccore * HID + k * 128
                        nc.sync.dma_start(htt[:, k, :ng * 128],
                                          HST[grp][rr:rr + 128, :])
                    for j in range(ng):
                        ps = psd_pool.tile([128, OUT], F32, tag="dense")
                        for k in range(K1):
                            nc.tensor.matmul(
                                ps[:], htt[:, k, j * 128:(j + 1) * 128],
                                wl2[:, k, :], start=(k == 0),
                                stop=(k == K1 - 1 and not p.use_bias))
                        if p.use_bias:
                            nc.tensor.matmul(ps[:], ones_t[:], blr2_t[:],
                                             start=False, stop=True)
                        o = eo_pool.tile([128, OUT], BF, tag="d2o")
                        if (ccore + j) % 2:
                            nc.vector.tensor_copy(o[:], ps[:])
                        else:
                            nc.scalar.copy(o[:], ps[:])
                        base = ccore * NOWN + (blk0 + j) * 128
                        nc.sync.dma_start(UL2S[base:base + 128, :], o[:])

            grp_of_blk = {}
            for gi, (b0, ng) in enumerate(AG_GROUPS):
                for b in range(b0, b0 + ng):
                    grp_of_blk[b] = (gi, b - b0)

            def epi1(k, vn, eo_pool, pss_pool):
                t = vn
                if p.use_bias:
                    t2 = eo_pool.tile([128, HID], F32, tag="t2")
                    nc.vector.tensor_tensor(t2[:], vn[:], b1rep_t[:],
                                            op=mybir.AluOpType.add)
                    t = t2
                h = eo_pool.tile([128, HID], BF, tag="h1")
                nc.scalar.activation(h[:], t[:],
                                     mybir.ActivationFunctionType.Relu)
                # transposed h -> hownT slice
                hT_ps = pss_pool.tile([128, HID], BF, tag="small")
                for kk in range(K1):
                    nc.tensor.matmul(hT_ps[:, kk * 128:(kk + 1) * 128],
                                     h[:, kk * 128:(kk + 1) * 128], id_t[:],
                                     is_transpose=True,
                                     start=(kk == 0), stop=(kk == K1 - 1))
                hT = eo_pool.tile([128, HID], BF, tag="hTs")
                nc.scalar.copy(hT[:], hT_ps[:])
                gi, joff = grp_of_blk[k]
                for kk in range(K1):
                    nc.sync.dma_start(
                        hownT[gi][kk * 128:(kk + 1) * 128,
                                  joff * 128:(joff + 1) * 128],
                        hT[:, kk * 128:(kk + 1) * 128])
                # ur2 for this block -> resident
                ps = psd_pool.tile([128, OUT], F32, tag="dense")
                for kk in range(K1):
                    nc.tensor.matmul(ps[:], hT[:, kk * 128:(kk + 1) * 128],
                                     wr2[:, kk, :], start=(kk == 0),
                                     stop=(kk == K1 - 1 and not p.use_bias))
                if p.use_bias:
                    nc.tensor.matmul(ps[:], ones_t[:], brr2_t[:],
                                     start=False, stop=True)
                nc.vector.tensor_copy(ur2_t[:, k, :], ps[:])
                for gi, (b0, ng) in enumerate(AG_GROUPS):
                    if k == b0 + ng - 1:
                        nc.gpsimd.collective_compute(
                            "AllGather", mybir.AluOpType.bypass,
                            replica_groups=[list(range(NCORES))],
                            ins=[hownT[gi][:].opt()],
                            outs=[HST[gi][:].opt()])
                # overlap: dense for an earlier gathered group
                if k == 5:
                    dense_ul2_group(0, eo_pool)
                if k == 9:
                    dense_ul2_group(1, eo_pool)

            # ---------------- layer 2 epilogue: layernorm -> out
            def epi2(k, vn, eo_pool, pss_pool):
                z = eo_pool.tile([128, OUT], F32, tag="z")
                nc.gpsimd.tensor_tensor(z[:], vn[:], inva2_t[:],
                                        op=mybir.AluOpType.mult)
                if p.use_bias:
                    nc.vector.tensor_tensor(z[:], z[:], b2rep_t[:],
                                            op=mybir.AluOpType.add)
                tmp = eo_pool.tile([128, OUT], F32, tag="lntmp")
                ssum = eo_pool.tile([128, 1], F32, tag="lnsum")
                nc.scalar.activation(tmp[:], z[:],
                                     mybir.ActivationFunctionType.Copy,
                                     accum_out=ssum[:])
                negmu = eo_pool.tile([128, 1], F32, tag="lnmu")
                nc.vector.tensor_scalar_mul(negmu[:], ssum[:], -1.0 / OUT)
                xm = eo_pool.tile([128, OUT], F32, tag="lnxm")
                nc.scalar.activation(xm[:], z[:],
                                     mybir.ActivationFunctionType.Identity,
                                     bias=negmu[:])
                sq = eo_pool.tile([128, OUT], F32, tag="lnsq")
                ssq = eo_pool.tile([128, 1], F32, tag="lnssq")
                nc.scalar.activation(sq[:], xm[:],
                                     mybir.ActivationFunctionType.Square,
                                     accum_out=ssq[:])
                sd = eo_pool.tile([128, 1], F32, tag="lnsd")
                nc.scalar.activation(sd[:], ssq[:],
                                     mybir.ActivationFunctionType.Sqrt,
                                     scale=1.0 / OUT, bias=eps_t[:])
                rstd = eo_pool.tile([128, 1], F32, tag="lnrstd")
                nc.vector.reciprocal(rstd[:], sd[:])
                og = eo_pool.tile([128, OUT], F32, tag="lnog")
                nc.vector.scalar_tensor_tensor(
                    og[:], xm[:], rstd[:], gam_t[:],
                    op0=mybir.AluOpType.mult, op1=mybir.AluOpType.mult)
                ob = eo_pool.tile([128, OUT], F32, tag="lnob")
                nc.gpsimd.tensor_tensor(ob[:], og[:], bet_t[:],
                                        op=mybir.AluOpType.add)
                nc.sync.dma_start(out_o[k * 128:(k + 1) * 128, :], ob[:])

            # ======== run layer 1
            with (
                tc.tile_pool(name="g1", bufs=3) as g_pool,
                tc.tile_pool(name="ew1", bufs=4) as ew_pool,
                tc.tile_pool(name="eo1", bufs=2) as eo_pool,
                tc.tile_pool(name="wt1", bufs=1) as wt_pool,
                tc.tile_pool(name="ps_t1", bufs=2, space="PSUM") as pst_pool,
                tc.tile_pool(name="ps_v1", bufs=1, space="PSUM") as psv_pool,
                tc.tile_pool(name="ps_e1", bufs=2, space="PSUM") as pse_pool,
                tc.tile_pool(name="ps_s1", bufs=2, space="PSUM") as pss_pool,
            ):
                pools = (g_pool, ew_pool, eo_pool, wt_pool,
                         pst_pool, psv_pool, pse_pool, pss_pool)
                edge_layer(1, ULtab, idx1_t, pools, HID, epi1)

            # ======== finish ul2 dense (group 2), then layer 2
            with (
                tc.tile_pool(name="g2", bufs=3) as g_pool,
                tc.tile_pool(name="ew2", bufs=4) as ew_pool,
                tc.tile_pool(name="eo2", bufs=2) as eo_pool,
                tc.tile_pool(name="wt2", bufs=1) as wt_pool,
                tc.tile_pool(name="ps_t2", bufs=2, space="PSUM") as pst_pool,
                tc.tile_pool(name="ps_v2", bufs=1, space="PSUM") as psv_pool,
                tc.tile_pool(name="ps_e2", bufs=2, space="PSUM") as pse_pool,
                tc.tile_pool(name="ps_s2", bufs=2, space="PSUM") as pss_pool,
            ):
                dense_ul2_group(2, eo_pool)
                pools = (g_pool, ew_pool, eo_pool, wt_pool,
                         pst_pool, psv_pool, pse_pool, pss_pool)
                edge_layer(2, UL2S, idx2_t, pools, OUT, epi2)

    nc.compile()
    return nc


# --------------------------------------------------------------- input maps

def _make_in_maps(p, inputs):
    N, F_IN, HID, OUT, H = p.N, p.F_IN, p.HID, p.OUT, p.H
    XPAD = ((N + 127) // 128) * 128
    NOWN = p.NBLK * 128
    x = np.asarray(inputs["x"], np.float32)
    xpad = np.zeros((XPAD, F_IN), np.float32)
    xpad[:N] = x
    xT = np.ascontiguousarray(xpad.T).astype(bf16)

    def attflat(att, Fo):
        H_, d = att.shape
        return np.asarray(att, np.float32).reshape(Fo)

    att1 = attflat(np.asarray(inputs["att1"], np.float32), HID)
    att2 = attflat(np.asarray(inputs["att2"], np.float32), OUT)
    a1 = np.abs(att1)
    a2 = np.abs(att2)
    tiny = np.float32(1e-20)
    a1 = np.maximum(a1, tiny)
    a2 = np.maximum(a2, tiny)
    sgn1 = np.sign(att1) + (att1 == 0)
    sgn2 = np.sign(att2) + (att2 == 0)

    def headsign(sgn, Fo):
        d = Fo // H
        S = np.zeros((Fo, H), np.float32)
        for h in range(H):
            S[h * d:(h + 1) * d, h] = sgn[h * d:(h + 1) * d]
        return S.astype(bf16)

    Wl1 = np.asarray(inputs["Wl1"], np.float32) * a1[:, None]
    Wr1 = np.asarray(inputs["Wr1"], np.float32) * a1[:, None]
    Wl2 = (np.asarray(inputs["Wl2"], np.float32) * a2[:, None]) / a1[None, :]
    Wr2 = (np.asarray(inputs["Wr2"], np.float32) * a2[:, None]) / a1[None, :]
    bl1 = np.asarray(inputs["bl1"], np.float32) * a1
    br1 = np.asarray(inputs["br1"], np.float32) * a1
    bl2 = np.asarray(inputs["bl2"], np.float32) * a2
    br2 = np.asarray(inputs["br2"], np.float32) * a2
    b1eff = np.asarray(inputs["bias1"], np.float32) * a1

    common = dict(
        xT=xT,
        WlT1=np.ascontiguousarray(Wl1.T).astype(bf16),
        WrT1=np.ascontiguousarray(Wr1.T).astype(bf16),
        WlT2=np.ascontiguousarray(Wl2.T).astype(bf16),
        WrT2=np.ascontiguousarray(Wr2.T).astype(bf16),
        S1d=headsign(sgn1, HID),
        S2d=headsign(sgn2, OUT),
        b1rep=np.broadcast_to(b1eff, (128, HID)).copy(),
        b2rep=np.broadcast_to(
            np.asarray(inputs["bias2"], np.float32), (128, OUT)).copy(),
        inva2r=np.broadcast_to(1.0 / a2, (128, OUT)).copy(),
        gam=np.broadcast_to(
            np.asarray(inputs["gamma"], np.float32), (128, OUT)).copy(),
        bet=np.broadcast_to(
            np.asarray(inputs["beta"], np.float32), (128, OUT)).copy(),
        ident=np.eye(128, dtype=np.float32).astype(bf16),
        identf=np.eye(4, dtype=np.float32),
        zr1=np.broadcast_to(-30.0 * sgn1, (128, HID)).astype(bf16).copy(),
        zr2=np.broadcast_to(-30.0 * sgn2, (128, OUT)).astype(bf16).copy(),
        id8d=np.kron(np.eye(4, dtype=np.float32),
                     np.ones((1, 2), np.float32)).astype(bf16),
        blr1=bl1.reshape(1, HID).astype(bf16),
        brr1=br1.reshape(1, HID).astype(bf16),
        blr2=bl2.reshape(1, OUT).astype(bf16),
        brr2=br2.reshape(1, OUT).astype(bf16),
    )
    in_maps = []
    for c in range(NCORES):
        m = dict(common)
        xo = np.zeros((NOWN, F_IN), np.float32)
        for k in range(p.NBLK):
            nodes = p.slot_nodes[(c, k)]
            xo[k * 128 + p.nslot[nodes]] = x[nodes]
        m["xownT"] = np.ascontiguousarray(xo.T).astype(bf16)
        m["idx1d"] = p.idx1[c]
        m["idx2d"] = p.idx2[c]
        in_maps.append(m)
    return in_maps


# ----------------------------------------------------------------- runner

class _Runner:
    def __init__(self, inputs):
        ei = np.asarray(inputs["edge_index"])
        N, F_IN = np.asarray(inputs["x"]).shape
        HID = np.asarray(inputs["Wl1"]).shape[0]
        OUT = np.asarray(inputs["Wl2"]).shape[0]
        H = np.asarray(inputs["att1"]).shape[0]
        self.eihash = hash(ei.tobytes())
        self.p = _Prep(N, ei.shape[1], F_IN, HID, OUT, H, ei)
        self.p.use_bias = any(
            np.abs(np.asarray(inputs[k])).max() > 0
            for k in ("bl1", "br1", "bl2", "br2", "bias1", "bias2"))
        self.nc = _build_nc(self.p)
        self.jit_fn = None

    def _prep_jit(self):
        """Build the shard_map jit once (mirrors bass2jax.run_bass_via_pjrt)."""
        import jax
        from jax.sharding import Mesh, PartitionSpec
        from jax.experimental.shard_map import shard_map
        from concourse import bass2jax
        from concourse.bass2jax import _bass_exec_p, partition_id_tensor
        nc = self.nc
        bass2jax.install_neuronx_cc_hook()
        pname = nc.partition_id_tensor.name if nc.partition_id_tensor else None
        in_names, out_names, out_avals, zero_outs = [], [], [], []
        for alloc in nc.m.functions[0].allocations:
            if not isinstance(alloc, mybir.MemoryLocationSet):
                continue
            name = alloc.memorylocations[0].name
            if alloc.kind == "ExternalInput":
                if name != pname:
                    in_names.append(name)
            elif alloc.kind == "ExternalOutput":
                out_names.append(name)
                shape = tuple(alloc.tensor_shape)
                dtype = mybir.dt.np(alloc.dtype)
                out_avals.append(jax.core.ShapedArray(shape, dtype))
                zero_outs.append(np.zeros(shape, dtype))
        n_params = len(in_names)
        all_names = in_names + out_names
        if pname is not None:
            all_names = all_names + [pname]

        def _body(*args):
            operands = list(args)
            if pname is not None:
                operands.append(partition_id_tensor())
            outs = _bass_exec_p.bind(
                *operands, out_avals=tuple(out_avals), in_names=tuple(all_names),
                out_names=tuple(out_names), lowering_input_output_aliases=(),
                sim_require_finite=True, sim_require_nnan=True, nc=nc)
            return tuple(outs)

        devices = jax.devices()[:NCORES]
        mesh = Mesh(np.asarray(devices), ("core",))
        n_outs = len(out_names)
        self.jit_fn = jax.jit(
            shard_map(_body, mesh=mesh,
                      in_specs=(PartitionSpec("core"),) * (n_params + n_outs),
                      out_specs=(PartitionSpec("core"),) * n_outs,
                      check_rep=False),
            keep_unused=True)
        self.in_names = in_names
        self.out_names = out_names
        self.out_avals = out_avals
        self.zero_outs = zero_outs
        self.mesh = mesh

    def device_args(self, inputs):
        in_maps = _make_in_maps(self.p, inputs)
        concat_in = [np.concatenate([in_maps[c][n] for c in range(NCORES)], 0)
                     for n in self.in_names]
        concat_zero = [np.zeros((NCORES * z.shape[0], *z.shape[1:]), z.dtype)
                       for z in self.zero_outs]
        return concat_in + concat_zero

    def run(self, inputs):
        if self.jit_fn is None:
            self._prep_jit()
        args = self.device_args(inputs)
        out_arrs = self.jit_fn(*args)
        res = [
            {n: np.asarray(out_arrs[i]).reshape(
                NCORES, *self.out_avals[i].shape)[c]
             for i, n in enumerate(self.out_names)}
            for c in range(NCORES)
        ]
        return self.assemble(res)

    def assemble(self, res):
        p = self.p
        out = np.zeros((p.N, p.OUT), np.float32)
        for c in range(NCORES):
            o = np.asarray(res[c]["out_o"], np.float32)
            for k in range(p.NBLK):
                nodes = p.slot_nodes[(c, k)]
                out[nodes] = o[k * 128 + p.nslot[nodes]]
        return out

    def timed_loop(self, inputs, r1=4, r2=40, reps=2):
        """Async-pipelined dispatch timing; difference two batch sizes to
        cancel fixed per-batch overhead."""
        import jax
        from jax.sharding import NamedSharding, PartitionSpec
        if self.jit_fn is None:
            self._prep_jit()
        args = self.device_args(inputs)
        sh = NamedSharding(self.mesh, PartitionSpec("core"))
        dargs = [jax.device_put(a, sh) for a in args]
        jax.block_until_ready(dargs)
        out = self.jit_fn(*dargs)
        jax.block_until_ready(out)

        def batch(R):
            ts = []
            for _ in range(reps):
                t0 = time.perf_counter()
                outs = [self.jit_fn(*dargs) for _ in range(R)]
                jax.block_until_ready(outs)
                ts.append(time.perf_counter() - t0)
            return min(ts)

        t1, t2 = batch(r1), batch(r2)
        return (t2 - t1) / (r2 - r1) * 1e9

    def timed(self, inputs, reps=5):
        import jax
        from jax.sharding import NamedSharding, PartitionSpec
        if self.jit_fn is None:
            self._prep_jit()
        args = self.device_args(inputs)
        sh = NamedSharding(self.mesh, PartitionSpec("core"))
        dargs = [jax.device_put(a, sh) for a in args]
        jax.block_until_ready(dargs)
        out = self.jit_fn(*dargs)
        jax.block_until_ready(out)
        times = []
        for _ in range(reps):
            t0 = time.perf_counter()
            out = self.jit_fn(*dargs)
            jax.block_until_ready(out)
            times.append(time.perf_counter() - t0)
        return min(times) * 1e9


_CACHE = {}


def kernel(**inputs):
    ei = np.asarray(inputs["edge_index"])
    key = hash(ei.tobytes())
    if key not in _CACHE:
        _CACHE.clear()
        _CACHE[key] = _Runner(inputs)
    r = _CACHE[key]
    try:
        return r.run(inputs)
    except Exception:
        from concourse.bass_utils import run_bass_kernel_spmd
        in_maps = _make_in_maps(r.p, inputs)
        res = run_bass_kernel_spmd(r.nc, in_maps, list(range(NCORES)))
        return r.assemble(res.results)
